# revision 1
# baseline (speedup 1.0000x reference)
"""Trainium2 Bass kernel for batched single-query attention (Luong-style).

  scores[b, t] = dec_hid[b] . enc_hid_states[b, t]      # [B, T]
  align        = softmax(scores, axis=1)
  c_t[b, d]    = sum_t align[b, t] * enc_hid_states[b, t, d]

Shapes: enc_hid_states [32, 8192, 256] f32, dec_hid [32, 256] f32.
Sharding: data-parallel over batch; 4 batches per core on 8 cores, no
cross-core communication (output rows are concatenated on the host).

Per-core pipeline (per batch, the 8 MiB enc slice is read from HBM exactly
once and kept in SBUF). Flash-attention style: each 1 MiB supertile
[128(t%128), 8(t//128), 256(d)] flows through a local softmax so every
engine is busy concurrently instead of phase-barriered:
  - DVE multiplies the supertile by a stride-0-broadcast dec vector;
    dot-product reduces split between DVE (3D tensor_reduce) and ACT
    (activation Copy + accum_out) to balance engine time
  - local max (DVE) -> GPSIMD partition all-reduce -> ACT Exp with
    bias=-m_s and fused sum-of-exp -> GPSIMD all-reduce
  - 8 accumulating PE matmuls (lhsT=probs column [128,1], rhs=enc tile
    [128,256], both fp16 = full-rate streaming) produce the supertile's
    partial context in PSUM
  - per batch, partials are combined with log-sum-exp weights
    w_s = exp(m_s - M): small PE transpose + matmuls (including the
    deferred cross-partition sum-of-exp reduce), scale by 1/Z.

enc/dec are cast f32->fp16 by the GPSIMD casting DMA on load: HBM traffic
is unchanged (32 MiB/core, read once) but the DVE multiply qualifies for
the all-2-byte 2x_1p perf mode (0.5x cycles) and SBUF footprint halves,
allowing 3 batches in flight. Cost: ~4e-3 relative error (vs ~8e-4 for
the all-f32 variant preserved in kernel_v2_flash_f32.py).

The kernel must avoid two environment pitfalls discovered empirically:
InstTensorTensorReduce faults this terminal's DVE (device becomes
NRT_EXEC_UNIT_UNRECOVERABLE), and the Tile kernel-tail semaphore
RANGE_CLEAR is replaced by a drain+barrier-only tail (see
_tail_no_semclear).
"""

import sys
from contextlib import ExitStack

import numpy as np

sys.path.insert(0, "/opt/trn_rl_repo")

import concourse.bacc as bacc
import concourse.bass as bass
import concourse.bass_isa as bass_isa
import concourse.mybir as mybir
import concourse.tile as tile
from concourse.bass_utils import run_bass_kernel_spmd
from concourse.tile import ScopedClock


def _tail_no_semclear(self, tick_clock, wait_clock):
    """Tile's kernel-tail normally drains, barriers, then issues a GPSIMD
    dma_reset + EVENT_SEMAPHORE_RANGE_CLEAR over every sem it allocated.
    NRT resets semaphore state between executions, so drain + barrier alone
    is sufficient under the one-shot PJRT execution used here."""
    drain_inst = self.nc.sync.drain()
    wait_clock.add_sem_waits(
        drain_inst.ins, ScopedClock({None: tick_clock.global_clock})
    )
    self.nc.all_engine_barrier()
    popped = self.nc._tile_sem_poison_stack.pop()
    assert popped is self._sem_poison


tile.TileContext._drain_and_barrier = _tail_no_semclear

B, T, D = 32, 8192, 256
N_CORES = 8
B_LOC = B // N_CORES  # 4 batches per core
P = 128               # partitions
NJ = T // P           # 64 row-tiles per batch
SUP = 8               # row-tiles per supertile (1 MiB DMA granularity)
NS = NJ // SUP        # 8 supertiles per batch
ST_BUFS = 30          # supertile slots, fp16 => 120 KiB/part (~4 batches)
DVE_REDUCE_SET = {0, 2, 4, 6}  # supertiles reduced on DVE; rest on ACT

# enc/probs live as fp16 on-chip: the GPSIMD casting DMA halves SBUF
# footprint, the all-2-byte DVE multiply runs in 2x_1p mode (0.5x cycles),
# and fp16 PE matmuls stream at 1 col/cycle like bf16.
PHASE2_DT = mybir.dt.float16


def _build_nc():
    f32 = mybir.dt.float32
    nc = bacc.Bacc(
        "TRN2",
        target_bir_lowering=False,
        debug=False,
        enable_asserts=False,
        num_devices=N_CORES,
    )
    enc = nc.dram_tensor("enc", [B_LOC, T, D], f32, kind="ExternalInput")
    dec = nc.dram_tensor("dec", [B_LOC, D], f32, kind="ExternalInput")
    out = nc.dram_tensor("out", [B_LOC, D], f32, kind="ExternalOutput")

    enc_r = enc.ap().rearrange("b (j p) d -> b p j d", p=P)  # [B_LOC, 128, 64, 256]
    dec_ap = dec.ap()
    out_ap = out.ap()

    with tile.TileContext(nc) as tc, ExitStack() as ctx:
        st_pool = ctx.enter_context(tc.tile_pool(name="st", bufs=ST_BUFS))
        prod_pool = ctx.enter_context(tc.tile_pool(name="prod", bufs=8))
        dec_pool = ctx.enter_context(tc.tile_pool(name="decb", bufs=2))
        small = ctx.enter_context(tc.tile_pool(name="small", bufs=8))
        outp = ctx.enter_context(tc.tile_pool(name="outp", bufs=2))
        psum_c = ctx.enter_context(tc.tile_pool(name="psc", bufs=4, space="PSUM"))
        psum_w = ctx.enter_context(tc.tile_pool(name="psw", bufs=1, space="PSUM"))

        # one-time constants
        ident1 = small.tile([1, 1], f32, tag="ident1")
        nc.vector.memset(ident1, 1.0)
        ones_col = small.tile([P, 1], f32, tag="ones_col")
        nc.vector.memset(ones_col, 1.0)

        for b in range(B_LOC):
            # dec[b] replicated across partitions and 8 j-groups
            dec_bc = dec_pool.tile([P, D], PHASE2_DT, tag="dec_bc")
            dslice = dec_ap[b : b + 1, :]
            dec_src = bass.AP(
                tensor=dslice.tensor,
                offset=dslice.offset,
                ap=[[0, P], [1, D]],
            )
            nc.gpsimd.dma_start(out=dec_bc, in_=dec_src)
            dec_bc3 = dec_bc[:, :].rearrange("p (u d) -> p u d", u=1).to_broadcast(
                [P, SUP, D]
            )

            sts = []
            for s in range(NS):
                st = st_pool.tile([P, SUP, D], PHASE2_DT, tag="st")
                nc.gpsimd.dma_start(
                    out=st,
                    in_=enc_r[b, :, s * SUP : (s + 1) * SUP, :],
                )
                sts.append(st)

            # per-supertile stats (column s of each is constant across
            # partitions after the GPSIMD all-reduce) and context partials
            SM = small.tile([P, NS], f32, tag="SM")    # local maxes
            SZ = small.tile([P, NS], f32, tag="SZ")    # per-partition sum-of-exp
            Csup = small.tile([NS, D], f32, tag="Csup")  # partial contexts

            for s in range(NS):
                # scores for this supertile
                S = small.tile([P, SUP], f32, tag="S")
                prod = prod_pool.tile([P, SUP, D], PHASE2_DT, tag="prod")
                nc.vector.tensor_tensor(
                    out=prod,
                    in0=sts[s],
                    in1=dec_bc3,
                    op=mybir.AluOpType.mult,
                )
                on_dve = s in DVE_REDUCE_SET
                if on_dve:
                    nc.vector.tensor_reduce(
                        out=S,
                        in_=prod,
                        axis=mybir.AxisListType.X,
                        op=mybir.AluOpType.add,
                    )
                else:
                    for jj in range(SUP):
                        junk = small.tile([P, D], PHASE2_DT, tag="junk")
                        nc.scalar.activation(
                            out=junk,
                            in_=prod[:, jj, :],
                            func=mybir.ActivationFunctionType.Copy,
                            bias=0.0,
                            scale=1.0,
                            accum_out=S[:, jj : jj + 1],
                        )

                # local softmax stats
                m_loc = small.tile([P, 1], f32, tag="m_loc")
                nc.vector.tensor_reduce(
                    out=m_loc, in_=S, axis=mybir.AxisListType.X,
                    op=mybir.AluOpType.max,
                )
                nc.gpsimd.partition_all_reduce(
                    SM[:, s : s + 1], m_loc, channels=P,
                    reduce_op=bass_isa.ReduceOp.max,
                )
                negm = small.tile([P, 1], f32, tag="negm")
                nc.gpsimd.tensor_scalar_mul(
                    out=negm, in0=SM[:, s : s + 1], scalar1=-1.0
                )

                probs = small.tile([P, SUP], PHASE2_DT, tag="probs")
                nc.scalar.activation(
                    out=probs,
                    in_=S,
                    func=mybir.ActivationFunctionType.Exp,
                    bias=negm,
                    scale=1.0,
                    accum_out=SZ[:, s : s + 1],
                )

                # partial context for this supertile
                ps = psum_c.tile([1, D], f32, tag="ps")
                for jj in range(SUP):
                    nc.tensor.matmul(
                        out=ps,
                        lhsT=probs[:, jj : jj + 1],
                        rhs=sts[s][:, jj, :],
                        start=(jj == 0),
                        stop=(jj == SUP - 1),
                    )
                # stage the partial at partition 0 (engines can't start at
                # partition s), then DMA it into row s of Csup
                csb = small.tile([1, D], f32, tag="csb")
                nc.vector.tensor_copy(out=csb, in_=ps)
                nc.sync.dma_start(out=Csup[s : s + 1, :], in_=csb)

            # combine: c = sum_s exp(m_s - M) * Csup[s] / sum_s exp(m_s - M) * Z_s
            M = small.tile([1, 1], f32, tag="M")
            nc.vector.tensor_reduce(
                out=M, in_=SM[0:1, :], axis=mybir.AxisListType.X,
                op=mybir.AluOpType.max,
            )
            negM = small.tile([1, 1], f32, tag="negM")
            nc.gpsimd.tensor_scalar_mul(out=negM, in0=M, scalar1=-1.0)
            w_row = small.tile([1, NS], f32, tag="w_row")
            nc.scalar.activation(
                out=w_row,
                in_=SM[0:1, :],
                func=mybir.ActivationFunctionType.Exp,
                bias=negM,
                scale=1.0,
            )
            # Z_col[s] = sum_p SZ[p, s] via PE, then Z = w . Z_col
            ps_z = psum_w.tile([NS, 1], f32, tag="ps_z")
            nc.tensor.matmul(
                out=ps_z, lhsT=SZ, rhs=ones_col, start=True, stop=True
            )
            z_col = small.tile([NS, 1], f32, tag="z_col")
            nc.vector.tensor_copy(out=z_col, in_=ps_z)

            # w as a column via PE transpose, then c_hat = w^T @ Csup
            ps_w = psum_w.tile([NS, 1], f32, tag="ps_w")
            nc.tensor.transpose(out=ps_w, in_=w_row, identity=ident1)
            w_col = small.tile([NS, 1], f32, tag="w_col")
            nc.vector.tensor_copy(out=w_col, in_=ps_w)
            ps_zf = psum_w.tile([1, 1], f32, tag="ps_zf")
            nc.tensor.matmul(
                out=ps_zf, lhsT=w_col, rhs=z_col, start=True, stop=True
            )
            invz = small.tile([1, 1], f32, tag="invz")
            nc.vector.reciprocal(out=invz, in_=ps_zf)
            ps_c = psum_w.tile([1, D], f32, tag="ps_chat")
            nc.tensor.matmul(
                out=ps_c, lhsT=w_col, rhs=Csup, start=True, stop=True
            )

            c_sb = outp.tile([1, D], f32, tag="c_sb")
            nc.vector.tensor_scalar_mul(out=c_sb, in0=ps_c, scalar1=invz)
            nc.sync.dma_start(out=out_ap[b : b + 1, :], in_=c_sb)

    nc.compile()
    return nc


_NC_CACHE = None


def _get_nc():
    global _NC_CACHE
    if _NC_CACHE is None:
        _NC_CACHE = _build_nc()
    return _NC_CACHE


def run_on_cores(enc_np: np.ndarray, dec_np: np.ndarray, trace: bool = False):
    """Returns (out [32, 256] f32, BassKernelResults)."""
    nc = _get_nc()
    in_maps = [
        {
            "enc": np.ascontiguousarray(enc_np[c * B_LOC : (c + 1) * B_LOC]),
            "dec": np.ascontiguousarray(dec_np[c * B_LOC : (c + 1) * B_LOC]),
        }
        for c in range(N_CORES)
    ]
    res = run_bass_kernel_spmd(nc, in_maps, list(range(N_CORES)), trace=trace)
    out = np.concatenate([r["out"] for r in res.results], axis=0)
    return out.astype(np.float32), res


def kernel(enc_hid_states, dec_hid):
    enc_np = np.asarray(enc_hid_states, dtype=np.float32)
    dec_np = np.asarray(dec_hid, dtype=np.float32)
    out, _ = run_on_cores(enc_np, dec_np, trace=False)
    return out



# revision 16
# speedup vs baseline: 1.5391x; 1.5391x over previous
"""Trainium2 Bass kernel for batched single-query attention (Luong-style).

  scores[b, t] = dec_hid[b] . enc_hid_states[b, t]      # [B, T]
  align        = softmax(scores, axis=1)
  c_t[b, d]    = sum_t align[b, t] * enc_hid_states[b, t, d]

Shapes: enc_hid_states [32, 8192, 256] f32, dec_hid [32, 256] f32.
Sharding: data-parallel over batch; 4 batches per core on 8 cores. The
host pre-casts both inputs to fp16 while sharding (numerically identical
to the on-device casting DMA the previous version used, and it frees the
GPSIMD engine for compute). Each core emits per-chunk softmax partials
(chunk = 2048 consecutive t of one batch); the host combines the 4
chunks of each batch with an exact log-sum-exp reduction in float64.

Per-core pipeline (v4). Each batch is split into 4 chunks of 2048 t laid
out [p=128 partitions, j=16, d=256] with t = p*16 + j inside the chunk,
so each partition's slice is one contiguous 8 KiB run in HBM (128 large
DMA descriptors per chunk). Per chunk, the scores dot-products are split
across three engines to match the DMA pace:
  - DVE: one 3D tensor_tensor multiplies 10 j-tiles by the broadcast dec
    (all-fp16 2x_1p mode), GPSIMD tensor_tensor multiplies the other 6
  - DVE: 13 tensor_scalar(+accum_out) ops reduce products over d into
    fp32 scores (4x_2p mode: the [P,1] accumulator is dtype-exempt);
    ACT Copy-with-accum reduces the other 3
  - chunk max: DVE tensor_reduce + GPSIMD partition all-reduce (max);
    ACT Exp with bias=-m writes fp16 probs and accumulates Z
  - 16 accumulating PE matmuls (lhsT=probs column [128,1], rhs=enc
    j-tile [128,256], fp16 full-rate) produce the chunk context in PSUM;
    ACT copies it to SBUF and the ACT queue DMAs it out unnormalized
Outputs per core: chat [16, 256] (unnormalized chunk contexts), szo
[128, 16] (per-partition sums of exp), nmo [1, 16] (minus chunk max).
Host: c[b] = sum_k w_k chat_k / sum_k w_k Z_k with w_k = exp(m_k - M).

Environment pitfalls avoided (discovered empirically on this device):
InstTensorTensorReduce faults the DVE (NRT_EXEC_UNIT_UNRECOVERABLE);
scalar_tensor_tensor gets no DVE perf modes in the cost model; InstPool
and TensorScalarPtr-with-accum are rejected on the Pool engine by
neuronxcc; plain tensor_scalar with accum_out needs op1/scalar2 set.
The Tile kernel-tail semaphore RANGE_CLEAR is replaced by a
drain+barrier-only tail (_tail_no_semclear).
"""

import sys
from contextlib import ExitStack

import numpy as np

sys.path.insert(0, "/opt/trn_rl_repo")

import concourse.bacc as bacc
import concourse.bass as bass
import concourse.bass_isa as bass_isa
import concourse.mybir as mybir
import concourse.tile as tile
from concourse.bass_utils import run_bass_kernel_spmd
from concourse.tile import ScopedClock


def _tail_no_semclear(self, tick_clock, wait_clock):
    """Tile's kernel-tail normally drains, barriers, then issues a GPSIMD
    dma_reset + EVENT_SEMAPHORE_RANGE_CLEAR over every sem it allocated.
    NRT resets semaphore state between executions, so drain + barrier alone
    is sufficient under the one-shot PJRT execution used here."""
    drain_inst = self.nc.sync.drain()
    wait_clock.add_sem_waits(
        drain_inst.ins, ScopedClock({None: tick_clock.global_clock})
    )
    self.nc.all_engine_barrier()
    popped = self.nc._tile_sem_poison_stack.pop()
    assert popped is self._sem_poison


tile.TileContext._drain_and_barrier = _tail_no_semclear

B, T, D = 32, 8192, 256
N_CORES = 8
B_LOC = B // N_CORES      # 4 batches per core
P = 128                   # partitions
NJB = T // P              # 64 j-tiles (128 t each) per batch

# chunk sizes in j-tiles per local batch, in schedule order: 1024-t chunks
# at the core's head and tail for fast pipeline fill/drain
BATCH_NJS = [
    [8, 8, 16, 16, 16],
    [16, 16, 16, 16],
    [16, 16, 16, 16],
    [16, 16, 16, 8, 8],
]
# (multiply j-tiles on GPSIMD, reduce j-tiles on ACT) per chunk size,
# from the engine-balance LP at each size's DMA-pace budget
SPLITS = {16: (5, 3), 8: (2, 1)}

CHUNKS = []  # (b, j0, nj)
for _b, _njs in enumerate(BATCH_NJS):
    assert sum(_njs) == NJB
    _j0 = 0
    for _nj in _njs:
        CHUNKS.append((_b, _j0, _nj))
        _j0 += _nj
NCH = len(CHUNKS)

F16 = mybir.dt.float16
F32 = mybir.dt.float32


def _build_nc():
    nc = bacc.Bacc(
        "TRN2",
        target_bir_lowering=False,
        debug=False,
        enable_asserts=False,
        num_devices=N_CORES,
    )
    enc = nc.dram_tensor("enc", [B_LOC, T, D], F16, kind="ExternalInput")
    dec = nc.dram_tensor("dec", [B_LOC, D], F16, kind="ExternalInput")
    chat = nc.dram_tensor("chat", [NCH, D], F32, kind="ExternalOutput")
    szo = nc.dram_tensor("szo", [P, NCH], F32, kind="ExternalOutput")
    nmo = nc.dram_tensor("nmo", [1, NCH], F32, kind="ExternalOutput")

    enc_ap = enc.ap()
    dec_ap = dec.ap()

    def enc_chunk_src(b, j0, nj):
        # [p, j, d] with t = j0*128 + p*nj + j: per-partition contiguous run
        return bass.AP(
            tensor=enc_ap.tensor,
            offset=enc_ap.offset + (b * T + j0 * P) * D,
            ap=[[nj * D, P], [D, nj], [1, D]],
        )

    with tile.TileContext(nc) as tc, ExitStack() as ctx:
        st_pool = ctx.enter_context(tc.tile_pool(name="st", bufs=8))
        prod_pool = ctx.enter_context(tc.tile_pool(name="prod", bufs=3))
        small = ctx.enter_context(tc.tile_pool(name="small", bufs=8))
        stats = ctx.enter_context(tc.tile_pool(name="stats", bufs=1))
        psum_c = ctx.enter_context(tc.tile_pool(name="psc", bufs=8, space="PSUM"))

        # dec[b] replicated across partitions, all 4 batches in one DMA
        # (issued after the first enc chunk so the bus starts on enc sooner)
        dec_all = stats.tile([P, B_LOC, D], F16, tag="dec_all")
        dec_src = bass.AP(
            tensor=dec_ap.tensor,
            offset=dec_ap.offset,
            ap=[[0, P], [D, B_LOC], [1, D]],
        )

        # persistent stats tiles (written per-chunk as columns)
        SZ = stats.tile([P, NCH], F32, tag="SZ")        # per-partition sum(exp)
        negSM = stats.tile([P, NCH], F32, tag="negSM")  # -chunk max (bcast)
        # rotating throwaway outputs for the reduce ops: a single shared
        # buffer would chain consecutive reduces through WAW semaphores
        junk_pool = ctx.enter_context(tc.tile_pool(name="junk", bufs=6))

        ps_t = {}

        def all_reduce_max(k):
            # deferred a chunk: folding the ACT-reduced score columns here
            # keeps the m-reduce off the ACT-copy critical path (the wait has
            # had a full chunk period to resolve)
            _, S, m1, nj, n_act = state[k]
            m2 = small.tile([P, 1], F32, tag="m2")
            nc.vector.tensor_reduce(
                out=m2, in_=S[:, 0:n_act], axis=mybir.AxisListType.X,
                op=mybir.AluOpType.max,
            )
            m = small.tile([P, 1], F32, tag="m")
            nc.vector.tensor_tensor(
                out=m, in0=m1, in1=m2, op=mybir.AluOpType.max
            )
            mar = small.tile([P, 1], F32, tag="mar")
            nc.gpsimd.partition_all_reduce(
                mar, m, channels=P, reduce_op=bass_isa.ReduceOp.max
            )
            nc.gpsimd.tensor_scalar_mul(
                out=negSM[:, k : k + 1], in0=mar, scalar1=-1.0
            )

        def exp_and_matmul(k):
            st, S, _, nj, _ = state[k]
            probs = small.tile([P, nj], F16, tag="probs")
            nc.scalar.activation(
                out=probs,
                in_=S,
                func=mybir.ActivationFunctionType.Exp,
                bias=negSM[:, k : k + 1],
                scale=1.0,
                accum_out=SZ[:, k : k + 1],
            )
            ps = psum_c.tile([1, D], F32, tag="ps")
            for j in range(nj):
                nc.tensor.matmul(
                    out=ps,
                    lhsT=probs[:, j : j + 1],
                    rhs=st[:, j, :],
                    start=(j == 0),
                    stop=(j == nj - 1),
                )
            ps_t[k] = ps

        def store_ctx(k):
            csb = small.tile([1, D], F32, tag="csb")
            nc.scalar.activation(
                out=csb, in_=ps_t[k],
                func=mybir.ActivationFunctionType.Copy, bias=0.0, scale=1.0,
            )
            nc.scalar.dma_start(out=chat.ap()[k : k + 1, :], in_=csb)

        state = {}
        for k, (b, j0, nj) in enumerate(CHUNKS):
            n_pool, n_act = SPLITS[nj]
            n_dve_mul = nj - n_pool
            st = st_pool.tile([P, nj, D], F16, tag="st")
            nc.sync.dma_start(out=st, in_=enc_chunk_src(b, j0, nj))
            if k == 0:
                nc.sync.dma_start(out=dec_all, in_=dec_src)

            dec_b3d = dec_all[:, b, :].rearrange("p (u d) -> p u d", u=1)
            prod_d = prod_pool.tile([P, n_dve_mul, D], F16, tag="prod_d")
            nc.vector.tensor_tensor(
                out=prod_d,
                in0=st[:, 0:n_dve_mul, :],
                in1=dec_b3d.to_broadcast([P, n_dve_mul, D]),
                op=mybir.AluOpType.mult,
            )
            prod_p = prod_pool.tile([P, n_pool, D], F16, tag="prod_p")
            nc.gpsimd.tensor_tensor(
                out=prod_p,
                in0=st[:, n_dve_mul:nj, :],
                in1=dec_b3d.to_broadcast([P, n_pool, D]),
                op=mybir.AluOpType.mult,
            )
            # all-reduce of the PREVIOUS chunk: issued after this chunk's
            # GPSIMD multiply so it never parks at the head of the in-order
            # Pool queue blocking that multiply (its m input is long ready)
            if k >= 1:
                all_reduce_max(k - 1)

            # ACT reduces the first j-tiles (from prod_d, ready earliest) so
            # its scores land before DVE's and the m-reduce never parks
            S = small.tile([P, nj], F32, tag="S")
            for j in range(n_act):
                junk_a = junk_pool.tile([P, D], F16, tag="junk_a")
                nc.scalar.activation(
                    out=junk_a, in_=prod_d[:, j, :],
                    func=mybir.ActivationFunctionType.Copy,
                    bias=0.0, scale=1.0,
                    accum_out=S[:, j : j + 1],
                )
            for j in range(n_act, nj):
                src = prod_d[:, j, :] if j < n_dve_mul else prod_p[:, j - n_dve_mul, :]
                junk_d = junk_pool.tile([P, D], F16, tag="junk_d")
                nc.vector.tensor_scalar(
                    out=junk_d, in0=src, scalar1=1.0, scalar2=0.0,
                    op0=mybir.AluOpType.mult, op1=mybir.AluOpType.add,
                    accum_out=S[:, j : j + 1],
                )

            m1 = small.tile([P, 1], F32, tag="m1")
            nc.vector.tensor_reduce(
                out=m1, in_=S[:, n_act:nj], axis=mybir.AxisListType.X,
                op=mybir.AluOpType.max,
            )
            state[k] = (st, S, m1, nj, n_act)

            # software pipelining, two chunks deep for the Exp: with a
            # one-chunk shift the pipeline is paced by the latency cycle
            # m(k) -> all-reduce -> Exp(k) -> [in-order ACT] -> score
            # copies(k+1) -> m(k+1); at two chunks the ACT queue reaches the
            # next chunk's score copies before the Exp that waits on the
            # all-reduce, and the cycle spans two periods instead of one
            if k >= 2:
                exp_and_matmul(k - 2)
            if k >= 3:
                store_ctx(k - 3)
        all_reduce_max(NCH - 1)
        exp_and_matmul(NCH - 2)
        exp_and_matmul(NCH - 1)
        store_ctx(NCH - 3)
        store_ctx(NCH - 2)
        store_ctx(NCH - 1)

        nc.scalar.dma_start(out=szo.ap(), in_=SZ)
        nc.scalar.dma_start(out=nmo.ap(), in_=negSM[0:1, :])

    nc.compile()
    return nc


_NC_CACHE = None


def _get_nc():
    global _NC_CACHE
    if _NC_CACHE is None:
        _NC_CACHE = _build_nc()
    return _NC_CACHE


def run_on_cores(enc_np: np.ndarray, dec_np: np.ndarray, trace: bool = False):
    """Returns (out [32, 256] f32, BassKernelResults)."""
    nc = _get_nc()
    enc16 = enc_np.astype(np.float16)
    dec16 = dec_np.astype(np.float16)
    in_maps = [
        {
            "enc": np.ascontiguousarray(enc16[c * B_LOC : (c + 1) * B_LOC]),
            "dec": np.ascontiguousarray(dec16[c * B_LOC : (c + 1) * B_LOC]),
        }
        for c in range(N_CORES)
    ]
    res = run_bass_kernel_spmd(nc, in_maps, list(range(N_CORES)), trace=trace)

    out = np.empty((B, D), dtype=np.float64)
    for c in range(N_CORES):
        r = res.results[c]
        chat = np.asarray(r["chat"], dtype=np.float64)   # [NCH, D]
        sz = np.asarray(r["szo"], dtype=np.float64)      # [P, NCH]
        mm = -np.asarray(r["nmo"], dtype=np.float64)[0]  # [NCH] chunk maxes
        z = sz.sum(axis=0)                               # [NCH]
        for lb in range(B_LOC):
            ks = [k for k, (bb, _, _) in enumerate(CHUNKS) if bb == lb]
            m_k, z_k, c_k = mm[ks], z[ks], chat[ks]
            M = m_k.max()
            w = np.exp(m_k - M)
            out[c * B_LOC + lb] = (w[:, None] * c_k).sum(0) / (w * z_k).sum()
    return out.astype(np.float32), res


def kernel(enc_hid_states, dec_hid):
    enc_np = np.asarray(enc_hid_states, dtype=np.float32)
    dec_np = np.asarray(dec_hid, dtype=np.float32)
    out, _ = run_on_cores(enc_np, dec_np, trace=False)
    return out


# revision 25
# speedup vs baseline: 1.5430x; 1.0025x over previous
"""Trainium2 Bass kernel for batched single-query attention (Luong-style).

  scores[b, t] = dec_hid[b] . enc_hid_states[b, t]      # [B, T]
  align        = softmax(scores, axis=1)
  c_t[b, d]    = sum_t align[b, t] * enc_hid_states[b, t, d]

Shapes: enc_hid_states [32, 8192, 256] f32, dec_hid [32, 256] f32.
Sharding: data-parallel over batch; 4 batches per core on 8 cores. The
host pre-casts both inputs to fp16 while sharding (numerically identical
to the on-device casting DMA the previous version used, and it frees the
GPSIMD engine for compute). Each core emits per-chunk softmax partials
(chunk = 2048 consecutive t of one batch); the host combines the 4
chunks of each batch with an exact log-sum-exp reduction in float64.

Per-core pipeline (v4). Each batch is split into 4 chunks of 2048 t laid
out [p=128 partitions, j=16, d=256] with t = p*16 + j inside the chunk,
so each partition's slice is one contiguous 8 KiB run in HBM (128 large
DMA descriptors per chunk). Per chunk, the scores dot-products are split
across three engines to match the DMA pace:
  - DVE: one 3D tensor_tensor multiplies 10 j-tiles by the broadcast dec
    (all-fp16 2x_1p mode), GPSIMD tensor_tensor multiplies the other 6
  - DVE: 13 tensor_scalar(+accum_out) ops reduce products over d into
    fp32 scores (4x_2p mode: the [P,1] accumulator is dtype-exempt);
    ACT Copy-with-accum reduces the other 3
  - chunk max: DVE tensor_reduce + GPSIMD partition all-reduce (max);
    ACT Exp with bias=-m writes fp16 probs and accumulates Z
  - 16 accumulating PE matmuls (lhsT=probs column [128,1], rhs=enc
    j-tile [128,256], fp16 full-rate) produce the chunk context in PSUM;
    ACT copies it to SBUF and the ACT queue DMAs it out unnormalized
Outputs per core: chat [16, 256] (unnormalized chunk contexts), szo
[128, 16] (per-partition sums of exp), nmo [1, 16] (minus chunk max).
Host: c[b] = sum_k w_k chat_k / sum_k w_k Z_k with w_k = exp(m_k - M).

Environment pitfalls avoided (discovered empirically on this device):
InstTensorTensorReduce faults the DVE (NRT_EXEC_UNIT_UNRECOVERABLE);
scalar_tensor_tensor gets no DVE perf modes in the cost model; InstPool
and TensorScalarPtr-with-accum are rejected on the Pool engine by
neuronxcc; plain tensor_scalar with accum_out needs op1/scalar2 set.
The Tile kernel-tail semaphore RANGE_CLEAR is replaced by a
drain+barrier-only tail (_tail_no_semclear).
"""

import sys
from contextlib import ExitStack

import numpy as np

sys.path.insert(0, "/opt/trn_rl_repo")

import concourse.bacc as bacc
import concourse.bass as bass
import concourse.bass_isa as bass_isa
import concourse.mybir as mybir
import concourse.tile as tile
from concourse.bass_utils import run_bass_kernel_spmd
from concourse.tile import ScopedClock


def _tail_no_semclear(self, tick_clock, wait_clock):
    """Tile's kernel-tail normally drains, barriers, then issues a GPSIMD
    dma_reset + EVENT_SEMAPHORE_RANGE_CLEAR over every sem it allocated.
    NRT resets semaphore state between executions, so drain + barrier alone
    is sufficient under the one-shot PJRT execution used here."""
    drain_inst = self.nc.sync.drain()
    wait_clock.add_sem_waits(
        drain_inst.ins, ScopedClock({None: tick_clock.global_clock})
    )
    self.nc.all_engine_barrier()
    popped = self.nc._tile_sem_poison_stack.pop()
    assert popped is self._sem_poison


tile.TileContext._drain_and_barrier = _tail_no_semclear

B, T, D = 32, 8192, 256
N_CORES = 8
B_LOC = B // N_CORES      # 4 batches per core
P = 128                   # partitions
NJB = T // P              # 64 j-tiles (128 t each) per batch

# chunk sizes in j-tiles per local batch, in schedule order: 1024-t chunks
# at the core's head and tail for fast pipeline fill/drain
BATCH_NJS = [
    [8, 8, 16, 16, 16],
    [16, 16, 16, 16],
    [16, 16, 16, 16],
    [16, 16, 16, 8, 8],
]
# (multiply j-tiles on GPSIMD, reduce j-tiles on ACT) per chunk size,
# from the engine-balance LP at each size's DMA-pace budget
SPLITS = {16: (5, 3), 8: (2, 1)}

CHUNKS = []  # (b, j0, nj)
for _b, _njs in enumerate(BATCH_NJS):
    assert sum(_njs) == NJB
    _j0 = 0
    for _nj in _njs:
        CHUNKS.append((_b, _j0, _nj))
        _j0 += _nj
NCH = len(CHUNKS)

F16 = mybir.dt.float16
F32 = mybir.dt.float32


def _build_nc():
    nc = bacc.Bacc(
        "TRN2",
        target_bir_lowering=False,
        debug=False,
        enable_asserts=False,
        num_devices=N_CORES,
    )
    enc = nc.dram_tensor("enc", [B_LOC, T, D], F16, kind="ExternalInput")
    dec = nc.dram_tensor("dec", [B_LOC, D], F16, kind="ExternalInput")
    chat = nc.dram_tensor("chat", [NCH, D], F32, kind="ExternalOutput")
    szo = nc.dram_tensor("szo", [P, NCH], F32, kind="ExternalOutput")
    nmo = nc.dram_tensor("nmo", [1, NCH], F32, kind="ExternalOutput")

    enc_ap = enc.ap()
    dec_ap = dec.ap()

    def enc_chunk_src(b, j0, nj):
        # [p, j, d] with t = j0*128 + p*nj + j: per-partition contiguous run
        return bass.AP(
            tensor=enc_ap.tensor,
            offset=enc_ap.offset + (b * T + j0 * P) * D,
            ap=[[nj * D, P], [D, nj], [1, D]],
        )

    with tile.TileContext(nc) as tc, ExitStack() as ctx:
        st_pool = ctx.enter_context(tc.tile_pool(name="st", bufs=8))
        prod_pool = ctx.enter_context(tc.tile_pool(name="prod", bufs=3))
        small = ctx.enter_context(tc.tile_pool(name="small", bufs=8))
        stats = ctx.enter_context(tc.tile_pool(name="stats", bufs=1))
        psum_c = ctx.enter_context(tc.tile_pool(name="psc", bufs=8, space="PSUM"))

        # dec[b] replicated across partitions, all 4 batches in one DMA
        # (issued after the first enc chunk so the bus starts on enc sooner)
        dec_all = stats.tile([P, B_LOC, D], F16, tag="dec_all")
        dec_src = bass.AP(
            tensor=dec_ap.tensor,
            offset=dec_ap.offset,
            ap=[[0, P], [D, B_LOC], [1, D]],
        )

        # persistent stats tiles (written per-chunk as columns)
        SZ = stats.tile([P, NCH], F32, tag="SZ")        # per-partition sum(exp)
        negSM = stats.tile([P, NCH], F32, tag="negSM")  # -chunk max (bcast)
        # rotating throwaway outputs for the reduce ops: a single shared
        # buffer would chain consecutive reduces through WAW semaphores
        junk_pool = ctx.enter_context(tc.tile_pool(name="junk", bufs=6))

        ps_t = {}

        def all_reduce_max(k):
            m = state[k][2]
            mar = small.tile([P, 1], F32, tag="mar")
            nc.gpsimd.partition_all_reduce(
                mar, m, channels=P, reduce_op=bass_isa.ReduceOp.max
            )
            nc.gpsimd.tensor_scalar_mul(
                out=negSM[:, k : k + 1], in0=mar, scalar1=-1.0
            )

        def exp_and_matmul(k):
            st, S, _, nj = state[k]
            probs = small.tile([P, nj], F16, tag="probs")
            nc.scalar.activation(
                out=probs,
                in_=S,
                func=mybir.ActivationFunctionType.Exp,
                bias=negSM[:, k : k + 1],
                scale=1.0,
                accum_out=SZ[:, k : k + 1],
            )
            ps = psum_c.tile([1, D], F32, tag="ps")
            for j in range(nj):
                nc.tensor.matmul(
                    out=ps,
                    lhsT=probs[:, j : j + 1],
                    rhs=st[:, j, :],
                    start=(j == 0),
                    stop=(j == nj - 1),
                )
            ps_t[k] = ps

        def store_ctx(k):
            csb = small.tile([1, D], F32, tag="csb")
            nc.scalar.activation(
                out=csb, in_=ps_t[k],
                func=mybir.ActivationFunctionType.Copy, bias=0.0, scale=1.0,
            )
            nc.scalar.dma_start(out=chat.ap()[k : k + 1, :], in_=csb)

        state = {}
        for k, (b, j0, nj) in enumerate(CHUNKS):
            n_pool, n_act = SPLITS[nj]
            n_dve_mul = nj - n_pool
            st = st_pool.tile([P, nj, D], F16, tag="st")
            nc.sync.dma_start(out=st, in_=enc_chunk_src(b, j0, nj))
            if k == 0:
                nc.sync.dma_start(out=dec_all, in_=dec_src)

            dec_b3d = dec_all[:, b, :].rearrange("p (u d) -> p u d", u=1)
            prod_d = prod_pool.tile([P, n_dve_mul, D], F16, tag="prod_d")
            nc.vector.tensor_tensor(
                out=prod_d,
                in0=st[:, 0:n_dve_mul, :],
                in1=dec_b3d.to_broadcast([P, n_dve_mul, D]),
                op=mybir.AluOpType.mult,
            )
            prod_p = prod_pool.tile([P, n_pool, D], F16, tag="prod_p")
            nc.gpsimd.tensor_tensor(
                out=prod_p,
                in0=st[:, n_dve_mul:nj, :],
                in1=dec_b3d.to_broadcast([P, n_pool, D]),
                op=mybir.AluOpType.mult,
            )
            # all-reduce of the PREVIOUS chunk: issued after this chunk's
            # GPSIMD multiply so it never parks at the head of the in-order
            # Pool queue blocking that multiply (its m input is long ready)
            if k >= 1:
                all_reduce_max(k - 1)

            # ACT reduces the first j-tiles (from prod_d, ready earliest) so
            # its scores land before DVE's and the m-reduce never parks
            S = small.tile([P, nj], F32, tag="S")
            for j in range(n_act):
                junk_a = junk_pool.tile([P, D], F16, tag="junk_a")
                nc.scalar.activation(
                    out=junk_a, in_=prod_d[:, j, :],
                    func=mybir.ActivationFunctionType.Copy,
                    bias=0.0, scale=1.0,
                    accum_out=S[:, j : j + 1],
                )
            for j in range(n_act, nj):
                if j < n_dve_mul:
                    src = prod_d[:, j, :]
                else:
                    src = prod_p[:, j - n_dve_mul, :]
                junk_d = junk_pool.tile([P, D], F16, tag="junk_d")
                nc.vector.tensor_scalar(
                    out=junk_d, in0=src, scalar1=1.0, scalar2=0.0,
                    op0=mybir.AluOpType.mult, op1=mybir.AluOpType.add,
                    accum_out=S[:, j : j + 1],
                )

            m = small.tile([P, 1], F32, tag="m")
            nc.vector.tensor_reduce(
                out=m, in_=S, axis=mybir.AxisListType.X, op=mybir.AluOpType.max
            )
            state[k] = (st, S, m, nj)

            # software pipelining, two chunks deep for the Exp: with a
            # one-chunk shift the pipeline is paced by the latency cycle
            # m(k) -> all-reduce -> Exp(k) -> [in-order ACT] -> score
            # copies(k+1) -> m(k+1); at two chunks the ACT queue reaches the
            # next chunk's score copies before the Exp that waits on the
            # all-reduce, and the cycle spans two periods instead of one
            if k >= 2:
                exp_and_matmul(k - 2)
            if k >= 3:
                store_ctx(k - 3)
        all_reduce_max(NCH - 1)
        exp_and_matmul(NCH - 2)
        exp_and_matmul(NCH - 1)
        store_ctx(NCH - 3)
        store_ctx(NCH - 2)
        store_ctx(NCH - 1)

        nc.scalar.dma_start(out=szo.ap(), in_=SZ)
        nc.scalar.dma_start(out=nmo.ap(), in_=negSM[0:1, :])

    nc.compile()
    return nc


_NC_CACHE = None


def _get_nc():
    global _NC_CACHE
    if _NC_CACHE is None:
        _NC_CACHE = _build_nc()
    return _NC_CACHE


def run_on_cores(enc_np: np.ndarray, dec_np: np.ndarray, trace: bool = False):
    """Returns (out [32, 256] f32, BassKernelResults)."""
    nc = _get_nc()
    enc16 = enc_np.astype(np.float16)
    dec16 = dec_np.astype(np.float16)
    in_maps = [
        {
            "enc": np.ascontiguousarray(enc16[c * B_LOC : (c + 1) * B_LOC]),
            "dec": np.ascontiguousarray(dec16[c * B_LOC : (c + 1) * B_LOC]),
        }
        for c in range(N_CORES)
    ]
    res = run_bass_kernel_spmd(nc, in_maps, list(range(N_CORES)), trace=trace)

    out = np.empty((B, D), dtype=np.float64)
    for c in range(N_CORES):
        r = res.results[c]
        chat = np.asarray(r["chat"], dtype=np.float64)   # [NCH, D]
        sz = np.asarray(r["szo"], dtype=np.float64)      # [P, NCH]
        mm = -np.asarray(r["nmo"], dtype=np.float64)[0]  # [NCH] chunk maxes
        z = sz.sum(axis=0)                               # [NCH]
        for lb in range(B_LOC):
            ks = [k for k, (bb, _, _) in enumerate(CHUNKS) if bb == lb]
            m_k, z_k, c_k = mm[ks], z[ks], chat[ks]
            M = m_k.max()
            w = np.exp(m_k - M)
            out[c * B_LOC + lb] = (w[:, None] * c_k).sum(0) / (w * z_k).sum()
    return out.astype(np.float32), res


def kernel(enc_hid_states, dec_hid):
    enc_np = np.asarray(enc_hid_states, dtype=np.float32)
    dec_np = np.asarray(dec_hid, dtype=np.float32)
    out, _ = run_on_cores(enc_np, dec_np, trace=False)
    return out
